# revision 14
# baseline (speedup 1.0000x reference)
"""BinaryLinear kernel for Trainium2 (8 NeuronCores, SPMD).

Computes  out = sign(x) @ sign(W)^T * alpha  for
x: [8192, 2048] f32, W: [2048, 2048] f32, alpha: [1] f32.

Strategy: data-parallel over tokens (8 shards of 1024); the weight is
split 8 ways over out_features for distribution. Every core reads
slices {0,1,2,3} of W^T as f32 locally (identical on all cores, so the
program stays SPMD-uniform) plus its OWN 256-col slice, which it signs
to fp8 and contributes to an HBM AllGather; slots {4..7} of the
AllGather output supply the remaining columns. This cuts per-core HBM
traffic from 32 MB to ~21 MB (vs. replicating all of W in f32).

Numerics: x is signed to +-0.5 in ONE DVE op ((x>0) - 0.5), W to +-1
via ACT sign; fp8(E4M3) holds both exactly, PSUM accumulates exact
half-integers |sum| <= 1024, and the drain scales by 2*alpha. Output is
written as f16 (integers up to 2048 are exact) and converted to f32 on
the host, halving output traffic.

Engine plan: ACT = W signs; DVE = x signs then PSUM drains; GpSimd
issues the (blocking) AllGather; PE runs 8-matmul DoubleRow units
(pass, m-tile) emitted in predicted-data-arrival order. Rings:
scalar = alpha + my-slice + x + outputs; sync = W f32 slices {0..3};
vector = fp8 bounce-out + AllGather slot loads.
"""

import numpy as np

import concourse.bass as bass
import concourse.tile as tile
from concourse import bacc, mybir
from concourse.bass_utils import run_bass_kernel_spmd

N_CORES = 8
NTOK = 8192
INF = 2048
OUTF = 2048
TPC = NTOK // N_CORES  # tokens per core (1024)
P = 128
KT = INF // P  # 16 contraction tiles
MT = TPC // P  # 8 token tiles per core
SL = OUTF // N_CORES  # 256 out_features per W slice
FD = 512  # matmul moving free dim (one PSUM bank)

F32 = mybir.dt.float32
F16 = mybir.dt.float16
FP8 = mybir.dt.float8e4

KC = 2  # k-tiles per w0123 f32 chunk (512 KB)
KSL = 4  # k-tiles per wsl f32 chunk (512 KB)

# pass -> (use local bwl?, col offset in rhs tile, col range in full output)
# pass0 = slices {0,1} (cols 0:512, local f32), pass3 = slices {2,3}
# (cols 512:1024, local f32), pass1 = AG slots {4,5} (cols 1024:1536),
# pass2 = AG slots {6,7} (cols 1536:2048).
PASS_LOCAL = {0: True, 3: True, 1: False, 2: False}
PASS_NOFF = {0: 0, 3: FD, 1: 0, 2: FD}

# Static PE emission order, sorted by predicted data readiness and
# grouped in same-pass m-pairs (so two drains share one 2KB-run out DMA):
# pass0 gated on w01 f32 + x m-arrival; pass3 on w23; passes 1/2 on the
# AllGather.
UNIT_ORDER = (
    [(0, 0), (0, 1), (0, 2), (0, 3), (0, 4), (0, 5)]
    + [(3, 0), (3, 1), (0, 6), (0, 7), (3, 2), (3, 3), (3, 4), (3, 5), (3, 6), (3, 7)]
    + [(1, m) for m in range(MT)]
    + [(2, m) for m in range(MT)]
)

_compiled = None
LAST_RESULT = None  # BassKernelResults of the most recent run (for profiling)


def _build():
    nc = bacc.Bacc(
        "TRN2",
        target_bir_lowering=False,
        debug=False,
        num_devices=N_CORES,
    )
    xt = nc.dram_tensor("xt", [MT * P * KT * P], F32, kind="ExternalInput").ap()
    wt = nc.dram_tensor("wt", [2 * KT * P * FD], F32, kind="ExternalInput").ap()
    wsl = nc.dram_tensor("wsl", [P * KT * SL], F32, kind="ExternalInput").ap()
    al = nc.dram_tensor("alpha", [P, 1], F32, kind="ExternalInput").ap()
    wsg_in = nc.dram_tensor("wsg_in", [P * KT * SL], FP8, kind="Internal")
    wsg_out = nc.dram_tensor(
        "wsg_out", [N_CORES * P * KT * SL], FP8, kind="Internal", addr_space="Shared"
    )
    out = nc.dram_tensor(
        "out", [4, MT // 2, P, 2 * FD], F16, kind="ExternalOutput"
    ).ap()

    with tile.TileContext(nc) as tc:
        with (
            tc.tile_pool(name="res", bufs=1) as res,
            tc.tile_pool(name="wload", bufs=5) as wload,
            tc.tile_pool(name="wsload", bufs=2) as wsload,
            tc.tile_pool(name="xload", bufs=3) as xload,
            tc.tile_pool(name="psum", bufs=8, space="PSUM") as ppool,
            tc.tile_pool(name="outp", bufs=4) as outp,
        ):
            bx = res.tile([P, KT, TPC], FP8)  # +-0.5 of x shard, 16 KB/part
            bwl = res.tile([P, KT, 2 * FD], FP8)  # slices 0-3, 16 KB/part
            bwr = res.tile([P, KT, 2 * FD], FP8)  # AG slots 4-7, 16 KB/part
            bsl = res.tile([P, KT, SL], FP8)  # my slice fp8, 4 KB/part
            alpha_t = res.tile([P, 1], F32)  # host-provided 2*alpha

            nc.scalar.dma_start(alpha_t[:], al)

            # -- my W slice: scalar ring f32 load -> ACT sign -> fp8 --
            for i in range(KT // KSL):
                wsf = wsload.tile([P, KSL, SL], F32, name="wsf", tag="wsf")
                src = wsl[i * P * KSL * SL : (i + 1) * P * KSL * SL].rearrange(
                    "(p f) -> p f", p=P
                )
                nc.scalar.dma_start(wsf[:].rearrange("p a b -> p (a b)"), src)
                nc.scalar.sign(bsl[:, i * KSL : (i + 1) * KSL, :], wsf[:])

            # -- bounce fp8 slice to HBM (scalar ring, right after wsl), then
            # AllGather (gpsimd engine; blocks it, nothing else there) --
            nc.scalar.dma_start(
                wsg_in.ap().rearrange("(p f) -> p f", p=P),
                bsl[:].rearrange("p a b -> p (a b)"),
            )
            nc.gpsimd.collective_compute(
                "AllGather",
                mybir.AluOpType.bypass,
                replica_groups=[list(range(N_CORES))],
                ins=[wsg_in.ap()],
                outs=[wsg_out.ap()],
            )


            # -- w0123 f32 (sync ring) -> ACT sign -> bwl --
            off = 0
            for pair in range(2):
                for kc in range(KT // KC):
                    wf = wload.tile([P, KC, FD], F32, name="wf", tag="wf")
                    src = wt[off : off + P * KC * FD].rearrange("(p f) -> p f", p=P)
                    nc.sync.dma_start(wf[:].rearrange("p a b -> p (a b)"), src)
                    nc.scalar.sign(
                        bwl[:, kc * KC : (kc + 1) * KC, pair * FD : (pair + 1) * FD],
                        wf[:],
                    )
                    off += P * KC * FD

            # AG slot loads (sync ring, behind w0123; gated on the AG): 4..7
            SLB = P * KT * SL
            for s in range(4, 8):
                src = wsg_out.ap()[s * SLB : (s + 1) * SLB].rearrange(
                    "(p f) -> p f", p=P
                )
                nc.sync.dma_start(bwr[:, :, (s - 4) * SL : (s - 3) * SL], src)

            # -- x m-chunks (scalar ring) -> DVE one-op sign to +-0.5 --
            for m in range(MT):
                xf = xload.tile([P, KT, P], F32, name="xf", tag="xf")
                src = xt[m * P * KT * P : (m + 1) * P * KT * P].rearrange(
                    "(p f) -> p f", p=P
                )
                nc.scalar.dma_start(xf[:].rearrange("p a b -> p (a b)"), src)
                nc.vector.tensor_scalar(
                    bx[:, :, m * P : (m + 1) * P], xf[:], 0.0, 0.5,
                    op0=mybir.AluOpType.is_gt, op1=mybir.AluOpType.subtract,
                )

            # -- PE units: 8 DoubleRow matmuls + DVE drain; same-pass m-pairs
            # share one out DMA (2KB runs) --
            ob2 = None
            for p, m in UNIT_ORDER:
                rhs_t = bwl if PASS_LOCAL[p] else bwr
                noff = PASS_NOFF[p]
                ps = ppool.tile([P, FD], F32, name="ps", tag="ps")
                for kc in range(KT // 2):
                    nc.tensor.matmul(
                        ps[:],
                        bx[:, 2 * kc : 2 * kc + 2, m * P : (m + 1) * P],
                        rhs_t[:, 2 * kc : 2 * kc + 2, noff : noff + FD],
                        start=(kc == 0),
                        stop=(kc == KT // 2 - 1),
                        perf_mode=mybir.MatmulPerfMode.DoubleRow,
                    )
                if m % 2 == 0:
                    ob2 = outp.tile([P, 2, FD], F16, name="ob", tag="ob")
                nc.vector.tensor_scalar_mul(ob2[:, m % 2, :], ps[:], alpha_t[:])
                if m % 2 == 1:
                    nc.scalar.dma_start(
                        out[p, m // 2], ob2[:].rearrange("p a b -> p (a b)")
                    )

    nc.compile()
    return nc


def _pack_common(weight):
    WT4 = np.ascontiguousarray(weight.T).reshape(KT, P, OUTF)
    parts = []
    for pair in range(2):
        cols = slice(pair * FD, (pair + 1) * FD)
        for kc in range(KT // KC):
            parts.append(WT4[kc * KC : (kc + 1) * KC, :, cols].transpose(1, 0, 2).ravel())
    wt_flat = np.ascontiguousarray(np.concatenate(parts))
    wsls = []
    for c in range(N_CORES):
        cols = slice(c * SL, (c + 1) * SL)
        ps = []
        for i in range(KT // KSL):
            ps.append(
                WT4[i * KSL : (i + 1) * KSL, :, cols].transpose(1, 0, 2).ravel()
            )
        wsls.append(np.ascontiguousarray(np.concatenate(ps)))
    return wt_flat, wsls


def _pack_x_shard(xs):
    xT4 = np.ascontiguousarray(xs.T).reshape(KT, P, TPC)
    return np.ascontiguousarray(
        np.concatenate(
            [xT4[:, :, m * P : (m + 1) * P].transpose(1, 0, 2).ravel() for m in range(MT)]
        )
    )


def kernel(x, weight, alpha):
    global _compiled, LAST_RESULT
    if _compiled is None:
        _compiled = _build()
    nc = _compiled

    x = np.asarray(x, dtype=np.float32)
    weight = np.asarray(weight, dtype=np.float32)
    alpha = np.asarray(alpha, dtype=np.float32)

    wt_flat, wsls = _pack_common(weight)
    alv = np.full((P, 1), 2.0 * float(alpha.reshape(-1)[0]), dtype=np.float32)
    in_maps = []
    for c in range(N_CORES):
        xs = _pack_x_shard(x[c * TPC : (c + 1) * TPC, :])
        in_maps.append({"xt": xs, "wt": wt_flat, "wsl": wsls[c], "alpha": alv})

    LAST_RESULT = run_bass_kernel_spmd(nc, in_maps, list(range(N_CORES)))
    full = np.empty((NTOK, OUTF), dtype=np.float32)
    # pass -> full-output column offset
    pass_cols = {0: 0, 3: FD, 1: 2 * FD, 2: 3 * FD}
    for c in range(N_CORES):
        o = LAST_RESULT.results[c]["out"]  # [4, MT//2, P, 2*FD] f16
        blk = (
            o.astype(np.float32)
            .reshape(4, MT // 2, P, 2, FD)
            .transpose(0, 1, 3, 2, 4)
            .reshape(4, TPC, FD)
        )
        for p in range(4):
            full[c * TPC : (c + 1) * TPC, pass_cols[p] : pass_cols[p] + FD] = blk[p]
    return full


# revision 22
# speedup vs baseline: 1.1544x; 1.1544x over previous
"""BinaryLinear kernel for Trainium2 (8 NeuronCores, SPMD).

Computes  out = sign(x) @ sign(W)^T * alpha  for
x: [8192, 2048] f32, W: [2048, 2048] f32, alpha: [1] f32.

Strategy: data-parallel over tokens (8 shards of 1024). Every core
reads W^T slices {0,1,2,3} as f32 locally (identical on all cores, so
the program stays SPMD-uniform) plus its OWN 256-col slice, which it
signs to fp8 and contributes to an HBM AllGather; slots {4..7} of the
AllGather output supply the remaining columns. Per-core HBM traffic
~21 MB vs 32 MB for full W replication.

Numerics: x tiles are signed to +-0.5 in one DVE op ((x>0) - 0.5) or
+-1 via ACT sign (late tiles, to unblock the DVE drain queue); W to
+-1 via ACT sign. fp8(E4M3) holds all exactly, PSUM sums are exact,
and each drain scales by 2*alpha or alpha per the tile's encoding.
Output is f16 (integers <= 2048 exact), converted to f32 on host.

DMA: three rings (scalar/Activation, sync/SP, gpsimd/SWDGE) balanced
by bytes, all major transfers with 4-8 KB per-partition runs. W01 is
split across scalar+sync so it lands early (it gates the first
matmul); w23 rides the gpsimd ring; the AllGather slot loads land in a
slot-major SBUF tile (contiguous 4 KB runs) consumed by FD-256
dual-accumulation-group DoubleRow units.
"""

import numpy as np

import concourse.bass as bass
import concourse.tile as tile
from concourse import bacc, mybir
from concourse.bass_utils import run_bass_kernel_spmd

N_CORES = 8
NTOK = 8192
INF = 2048
OUTF = 2048
TPC = NTOK // N_CORES  # tokens per core (1024)
P = 128
KT = INF // P  # 16 contraction tiles
MT = TPC // P  # 8 token tiles per core
SL = OUTF // N_CORES  # 256 out_features per W slice
FD = 512  # PSUM bank free dim

F32 = mybir.dt.float32
F16 = mybir.dt.float16
FP8 = mybir.dt.float8e4

# x tiles signed on ACT (+-1) instead of DVE (+-0.5). Empty for now: all
# x tiles go through the one-op DVE sign.
ACT_X_TILES = ()

# Static PE emission order over fine-grained (slice, m) units, sorted
# by predicted data readiness. Slices 0-3 come from local f32 (w01 then
# w23), slices 4-7 from the AllGather. Each unit is one FD-256
# DoubleRow accumulation group in its own PSUM bank; 8 consecutive
# units share one 4KB-run out DMA.
UNIT_ORDER = (
    [(0, 0), (1, 0), (0, 1), (1, 1), (0, 2), (1, 2), (0, 3), (1, 3)]
    + [(2, 0), (3, 0), (0, 4), (1, 4), (2, 1), (3, 1), (0, 5), (1, 5)]
    + [(2, 2), (3, 2), (0, 6), (1, 6), (2, 3), (3, 3), (0, 7), (1, 7)]
    + [(2, 4), (3, 4), (2, 5), (3, 5), (2, 6), (3, 6), (2, 7), (3, 7)]
    + [(4, 0), (5, 0), (4, 1), (5, 1), (4, 2), (5, 2), (4, 3), (5, 3)]
    + [(4, 4), (5, 4), (4, 5), (5, 5), (4, 6), (5, 6), (4, 7), (5, 7)]
    + [(6, 0), (7, 0), (6, 1), (7, 1), (6, 2), (7, 2), (6, 3), (7, 3)]
    + [(6, 4), (7, 4), (6, 5), (7, 5), (6, 6), (7, 6), (6, 7), (7, 7)]
)

_compiled = None
LAST_RESULT = None  # BassKernelResults of the most recent run (for profiling)


def _build():
    nc = bacc.Bacc(
        "TRN2",
        target_bir_lowering=False,
        debug=False,
        num_devices=N_CORES,
    )
    xt = nc.dram_tensor("xt", [MT * P * KT * P], F32, kind="ExternalInput").ap()
    # w01: pair {0,1} in 4 k-quad chunks [128,4,512]; w23 likewise
    w01 = nc.dram_tensor("w01", [KT * P * FD], F32, kind="ExternalInput").ap()
    w23 = nc.dram_tensor("w23", [KT * P * FD], F32, kind="ExternalInput").ap()
    # my slice, 2 chunks [128,8,256]
    wsl = nc.dram_tensor("wsl", [P * KT * SL], F32, kind="ExternalInput").ap()
    al = nc.dram_tensor("alpha", [P, 2], F32, kind="ExternalInput").ap()
    wsg_in = nc.dram_tensor("wsg_in", [P * KT * SL], FP8, kind="Internal")
    wsg_out = nc.dram_tensor(
        "wsg_out", [N_CORES * P * KT * SL], FP8, kind="Internal", addr_space="Shared"
    )
    out = nc.dram_tensor(
        "out", [8, P, 8 * SL], F16, kind="ExternalOutput"
    ).ap()

    with tile.TileContext(nc) as tc:
        with (
            tc.tile_pool(name="res", bufs=1) as res,
            tc.tile_pool(name="wload", bufs=4) as wload,
            tc.tile_pool(name="wsload", bufs=2) as wsload,
            tc.tile_pool(name="xload", bufs=4) as xload,
            tc.tile_pool(name="psum", bufs=8, space="PSUM") as ppool,
            tc.tile_pool(name="outp", bufs=3) as outp,
        ):
            bx = res.tile([P, KT, TPC], FP8)  # x signs, 16 KB/part
            bwl = res.tile([P, KT, 2 * FD], FP8)  # slices 0-3, 16 KB/part
            # AG slots 4-7, slot-major so loads are contiguous 4KB runs
            bwr = res.tile([P, 4, KT, SL], FP8)  # 16 KB/part
            bsl = res.tile([P, KT, SL], FP8)  # my slice fp8, 4 KB/part
            alpha_t = res.tile([P, 2], F32)  # [2*alpha, alpha] from host

            nc.scalar.dma_start(alpha_t[:], al)

            # emission helpers ------------------------------------------------
            def w01_chunk(ring, kq):
                wf = wload.tile([P, 4, FD], F32, name="wf", tag="wf")
                src = w01[kq * P * 4 * FD : (kq + 1) * P * 4 * FD].rearrange(
                    "(p f) -> p f", p=P
                )
                ring.dma_start(wf[:].rearrange("p a b -> p (a b)"), src)
                nc.scalar.sign(bwl[:, kq * 4 : (kq + 1) * 4, 0:FD], wf[:])

            def w23_chunk(ring, kq):
                wf = wload.tile([P, 4, FD], F32, name="wf", tag="wf")
                src = w23[kq * P * 4 * FD : (kq + 1) * P * 4 * FD].rearrange(
                    "(p f) -> p f", p=P
                )
                ring.dma_start(wf[:].rearrange("p a b -> p (a b)"), src)
                nc.scalar.sign(bwl[:, kq * 4 : (kq + 1) * 4, FD : 2 * FD], wf[:])

            def x_chunk(ring, m):
                xf = xload.tile([P, KT, P], F32, name="xf", tag="xf")
                src = xt[m * P * KT * P : (m + 1) * P * KT * P].rearrange(
                    "(p f) -> p f", p=P
                )
                ring.dma_start(xf[:].rearrange("p a b -> p (a b)"), src)
                if m in ACT_X_TILES:
                    nc.scalar.sign(bx[:, :, m * P : (m + 1) * P], xf[:])
                else:
                    nc.vector.tensor_scalar(
                        bx[:, :, m * P : (m + 1) * P], xf[:], 0.0, 0.5,
                        op0=mybir.AluOpType.is_gt, op1=mybir.AluOpType.subtract,
                    )

            def wsl_chunk(ring, i):
                wsf = wsload.tile([P, 8, SL], F32, name="wsf", tag="wsf")
                src = wsl[i * P * 8 * SL : (i + 1) * P * 8 * SL].rearrange(
                    "(p f) -> p f", p=P
                )
                ring.dma_start(wsf[:].rearrange("p a b -> p (a b)"), src)
                nc.scalar.sign(bsl[:, i * 8 : (i + 1) * 8, :], wsf[:])

            # ring schedules --------------------------------------------------
            # scalar: w01 quads 0,1 | x0 x2 x4 x6 | outs-even (in unit loop)
            # sync:   w01 quads 2,3 | x1 | wsl | bounce | x3 x5 x7 | outs-odd
            # gpsimd: w23 halves, bounce slot is mid-queue; engine then issues
            #         the blocking AG, then agload descs fire on its ring.
            w01_chunk(nc.scalar, 0)
            w01_chunk(nc.sync, 2)
            w01_chunk(nc.scalar, 1)
            w01_chunk(nc.sync, 3)
            w23_chunk(nc.gpsimd, 0)
            w23_chunk(nc.gpsimd, 1)
            x_chunk(nc.scalar, 0)
            x_chunk(nc.sync, 1)
            wsl_chunk(nc.sync, 0)
            wsl_chunk(nc.sync, 1)
            nc.gpsimd.dma_start(
                wsg_in.ap().rearrange("(p f) -> p f", p=P),
                bsl[:].rearrange("p a b -> p (a b)"),
            )
            w23_chunk(nc.gpsimd, 2)
            w23_chunk(nc.gpsimd, 3)
            x_chunk(nc.scalar, 2)
            x_chunk(nc.sync, 3)
            x_chunk(nc.scalar, 4)
            x_chunk(nc.sync, 5)
            x_chunk(nc.scalar, 6)
            x_chunk(nc.sync, 7)

            nc.gpsimd.collective_compute(
                "AllGather",
                mybir.AluOpType.bypass,
                replica_groups=[list(range(N_CORES))],
                ins=[wsg_in.ap()],
                outs=[wsg_out.ap()],
            )
            # AG slot loads (gpsimd ring tail, gated on the AG): slots 4..7
            SLB = P * KT * SL
            for s in range(4, 8):
                src = wsg_out.ap()[s * SLB : (s + 1) * SLB].rearrange(
                    "(p f) -> p f", p=P
                )
                nc.gpsimd.dma_start(
                    bwr[:, s - 4, :, :].rearrange("p a b -> p (a b)"), src
                )

            # -- PE units ----------------------------------------------------
            # one FD-256 DoubleRow accumulation group per unit, in its own
            # PSUM bank (allocated [P, FD] so banks are never shared).
            ob8 = None
            for ui, (s, m) in enumerate(UNIT_ORDER):
                ps = ppool.tile([P, FD], F32, name="ps", tag="ps")
                if s < 4:
                    rhs = lambda kc: bwl[:, 2 * kc : 2 * kc + 2, s * SL : (s + 1) * SL]
                else:
                    rhs = lambda kc: bwr[:, s - 4, 2 * kc : 2 * kc + 2, :]
                for kc in range(KT // 2):
                    nc.tensor.matmul(
                        ps[:, 0:SL],
                        bx[:, 2 * kc : 2 * kc + 2, m * P : (m + 1) * P],
                        rhs(kc),
                        start=(kc == 0),
                        stop=(kc == KT // 2 - 1),
                        perf_mode=mybir.MatmulPerfMode.DoubleRow,
                    )
                if ui % 8 == 0:
                    ob8 = outp.tile([P, 8, SL], F16, name="ob", tag="ob")
                acol = 1 if m in ACT_X_TILES else 0
                nc.vector.tensor_scalar_mul(
                    ob8[:, ui % 8, :], ps[:, 0:SL], alpha_t[:, acol : acol + 1]
                )
                if ui % 8 == 7:
                    ring = nc.scalar if (ui // 8) % 2 == 0 else nc.sync
                    ring.dma_start(
                        out[ui // 8], ob8[:].rearrange("p a b -> p (a b)")
                    )

    nc.compile()
    return nc


def _pack_common(weight):
    WT4 = np.ascontiguousarray(weight.T).reshape(KT, P, OUTF)
    w01 = np.concatenate(
        [
            WT4[kq * 4 : (kq + 1) * 4, :, 0:FD].transpose(1, 0, 2).ravel()
            for kq in range(4)
        ]
    )
    w23 = np.concatenate(
        [
            WT4[kq * 4 : (kq + 1) * 4, :, FD : 2 * FD].transpose(1, 0, 2).ravel()
            for kq in range(4)
        ]
    )
    wsls = []
    for c in range(N_CORES):
        cols = slice(c * SL, (c + 1) * SL)
        wsls.append(
            np.ascontiguousarray(
                np.concatenate(
                    [
                        WT4[i * 8 : (i + 1) * 8, :, cols].transpose(1, 0, 2).ravel()
                        for i in range(2)
                    ]
                )
            )
        )
    return np.ascontiguousarray(w01), np.ascontiguousarray(w23), wsls


def _pack_x_shard(xs):
    xT4 = np.ascontiguousarray(xs.T).reshape(KT, P, TPC)
    return np.ascontiguousarray(
        np.concatenate(
            [xT4[:, :, m * P : (m + 1) * P].transpose(1, 0, 2).ravel() for m in range(MT)]
        )
    )


def kernel(x, weight, alpha):
    global _compiled, LAST_RESULT
    if _compiled is None:
        _compiled = _build()
    nc = _compiled

    x = np.asarray(x, dtype=np.float32)
    weight = np.asarray(weight, dtype=np.float32)
    alpha = np.asarray(alpha, dtype=np.float32)

    w01, w23, wsls = _pack_common(weight)
    a = float(alpha.reshape(-1)[0])
    alv = np.empty((P, 2), dtype=np.float32)
    alv[:, 0] = 2.0 * a
    alv[:, 1] = a
    in_maps = []
    for c in range(N_CORES):
        xs = _pack_x_shard(x[c * TPC : (c + 1) * TPC, :])
        in_maps.append(
            {"xt": xs, "w01": w01, "w23": w23, "wsl": wsls[c], "alpha": alv}
        )

    LAST_RESULT = run_bass_kernel_spmd(nc, in_maps, list(range(N_CORES)))
    full = np.empty((NTOK, OUTF), dtype=np.float32)
    for c in range(N_CORES):
        o = LAST_RESULT.results[c]["out"].astype(np.float32)  # [8, P, 8*SL]
        o = o.reshape(8, P, 8, SL)
        for ui, (s, m) in enumerate(UNIT_ORDER):
            rows = slice(c * TPC + m * P, c * TPC + (m + 1) * P)
            cols = slice(s * SL, (s + 1) * SL)
            full[rows, cols] = o[ui // 8, :, ui % 8, :]
    return full
